# revision 45
# baseline (speedup 1.0000x reference)
"""Trainium2 Bass kernel for CINConv-style GNN message passing.

Strategy (8 NeuronCores, data parallel over destination nodes):
  - Nodes are packed into (core, 128-node block) bins on the host with a
    multi-dim best-fit so per-(type, src-half) block edge counts stay under
    aligned multiples of 128 across all cores: gather chunks run nearly
    full and the SPMD schedule (max over cores) has little padding.
  - Edges are bucketed by (dst core, type, dst block, src half). Source
    rows are gathered from a bf16 copy of x with `dma_gather` (Q7 SWDGE,
    int16 indices -> x addressed as two halves), 128 edges per chunk, in
    ~16-chunk pieces rotated over all 4 SWDGE queues (the gather is the
    roofline: ~0.47 rows/ns; pieces must stay inside the descriptor ring).
    Pad slots cycle real row indices so they don't hotspot one DRAM page.
  - Aggregation happens on the PE with one-hot matmuls into PSUM:
        agg^T[d, n] += feat[e, d]^T @ onehot[e, n]
    The one-hot is built on DVE via is_equal with a x2-packed last dim
    (dst offsets duplicated) to engage the 16-bit 2x/4x DVE mode.
  - Self terms ((1+eps)*x) ride as identity matmuls; x^T for the upper
    branch is DMA'd pre-transposed from the host.
  - Per-node MLPs run in transposed layout as float32r matmuls (1 cycle/
    row at >=256 free size); second-layer weights are host-fused into the
    output projection. Results are PE-transposed back and stored.
  - Per-super-block gidx/dst slices load just-in-time on the Activation
    HWDGE queue; a dummy warmup gather absorbs one-time SWDGE setup.
"""

import numpy as np
import ml_dtypes

import concourse.bass as bass
import concourse.mybir as mybir
from concourse import bacc
from concourse.tile import TileContext
from concourse.bass_utils import run_bass_kernel_spmd

bf16 = ml_dtypes.bfloat16
F32 = mybir.dt.float32
BF16 = mybir.dt.bfloat16
I16 = mybir.dt.int16

# ---- problem config (hardcoded) ----
N, E, D = 50000, 800000, 128
NC = 8
BLK = 128
PAD_DST = 200.0
SPLIT = 32768  # int16 index limit: x rows addressed as [0,SPLIT) + [SPLIT,N)
BOUNDARY, UPPER, REWIRE = 0, 1, 2
NT = 4  # chunk types: B, R, U1 (src half of upper msg), U2 (upper_ind half)

LAST_EXEC_NS = None
LAST_TRACE_PATH = None


def _cfg(n, n_cores):
    shard = n // n_cores
    nblk = -(-shard // BLK)
    return shard, nblk, nblk * BLK


# ---------------------------------------------------------------- host prep
def node_degrees(src, dst, et, ui, n):
    """Per-node 8-dim gather-ref counts: (t, src-half) for t in B,R,U1,U2."""
    d8 = np.zeros((n, 8), np.int32)
    streams = [(0, src[et == BOUNDARY], dst[et == BOUNDARY]),
               (1, src[et == REWIRE], dst[et == REWIRE]),
               (2, src[et == UPPER], dst[et == UPPER]),
               (3, ui[et == UPPER], dst[et == UPPER])]
    for t, rows, dn in streams:
        h = (rows >= SPLIT).astype(np.int64)
        np.add.at(d8, (dn, t * 2 + h), 1)
    return d8


def assign_nodes(d8, n_cores, nblk):
    """Pack nodes into (core, block, slot) so per-(type,half) block sums stay
    under aligned multiples of 128 (fewer, fuller gather chunks, balanced
    across cores).  Returns slot2node [n_cores, nblk*BLK] (-1 = pad)."""
    n = d8.shape[0]
    tot = d8.sum(1)
    order = np.argsort(-tot, kind="stable")
    core_of = np.zeros(n, np.int64)
    for i, nd in enumerate(order):
        r, j = divmod(i, n_cores)
        core_of[nd] = j if (r % 2 == 0) else n_cores - 1 - j

    # global per-dim chunk budget (max over cores, plus slack so discrete
    # packing can succeed) -> per-block caps at multiples of 128
    slack = 14
    q = np.zeros((n_cores, 8), np.int64)
    for c in range(n_cores):
        q[c] = -(-d8[core_of == c].sum(0) // BLK)
    qg = q.max(0) + slack
    base = qg // nblk
    n_hi = qg - base * nblk
    caps = np.zeros((nblk, 8), np.int64)
    off = 0
    for dd in range(8):
        caps[:, dd] = base[dd] * BLK
        hi = (np.arange(off, off + n_hi[dd]) % nblk)
        caps[hi, dd] += BLK
        off += int(n_hi[dd]) * 7 // 2 + 11
    slot2node = np.full((n_cores, nblk * BLK), -1, np.int64)
    for c in range(n_cores):
        ids = np.where(core_of == c)[0]
        ids = ids[np.argsort(-tot[ids], kind="stable")]
        vecs = d8[ids]
        S = np.zeros((nblk, 8), np.int64)
        cnt = np.zeros(nblk, np.int64)
        bins = [[] for _ in range(nblk)]
        # best-fit decreasing into all blocks simultaneously: place each node
        # where it fits with the most balanced remaining headroom
        for j in range(len(ids)):
            v = vecs[j]
            head = caps - S - v[None, :]
            open_ = cnt < BLK
            feas = open_ & (head >= 0).all(1)
            if feas.any():
                score = head.min(1).astype(np.float64) + 0.001 * (BLK - cnt)
                score[~feas] = -np.inf
                b = int(np.argmax(score))
            else:
                over = np.maximum(-head, 0).sum(1).astype(np.float64)
                over[~open_] = np.inf
                b = int(np.argmin(over))
            bins[b].append(j)
            S[b] += v
            cnt[b] += 1
        for b in range(nblk):
            slot2node[c, b * BLK:b * BLK + len(bins[b])] = ids[bins[b]]
    return refine_assignment(slot2node, d8, n_cores, nblk)


def refine_assignment(slot2node, d8, n_cores, nblk):
    """Local search: where a (dim, block) count barely crosses a 128
    boundary on some core, swap nodes between that core's blocks to pull it
    under — each success saves one shared gather chunk."""
    blk_of = np.repeat(np.arange(nblk), BLK)
    for _round in range(8):
        # per-core per-block dim sums
        S = np.zeros((n_cores, nblk, 8), np.int64)
        for c in range(n_cores):
            s2n = slot2node[c]
            v = s2n >= 0
            np.add.at(S[c], blk_of[v], d8[s2n[v]])
        mx = S.max(0)                       # [nblk, 8]
        k = -(-mx // BLK)
        base = (k - 1) * BLK                # lower boundary
        r = mx - base                       # 1..128; small r = cheap to fix
        improved = 0
        targets = [(int(r[b, dd]), b, dd) for b in range(nblk)
                   for dd in range(8) if 0 < r[b, dd] <= 90 and k[b, dd] > 1]
        targets.sort()
        for _r, b, dd in targets:
            for c in range(n_cores):
                need = S[c, b, dd] - base[b, dd]
                if need <= 0:
                    continue
                # move dim-dd weight out of (c,b) via swaps
                slots_b = np.arange(b * BLK, (b + 1) * BLK)
                nb_ = slot2node[c, slots_b]
                cand = slots_b[(nb_ >= 0)]
                cand = cand[np.argsort(-d8[slot2node[c, cand], dd])]
                for sl in cand:
                    if need <= 0:
                        break
                    nd = slot2node[c, sl]
                    w = d8[nd]
                    if w[dd] == 0 or w[dd] > need + 8:
                        continue
                    # find a partner block with headroom and a 0-weight node
                    done = False
                    for b2 in np.argsort(S[c, :, dd]):
                        if b2 == b:
                            continue
                        cap2 = k[b2] * BLK
                        if ((S[c, b2] + w) > cap2).any():
                            continue
                        slots2 = np.arange(b2 * BLK, (b2 + 1) * BLK)
                        nb2 = slot2node[c, slots2]
                        ok2 = slots2[nb2 >= 0]
                        for sl2 in ok2:
                            nd2 = slot2node[c, sl2]
                            w2 = d8[nd2]
                            if w2[dd] >= w[dd]:
                                continue
                            newb = S[c, b] - w + w2
                            if (newb > k[b] * BLK).any():
                                continue
                            if ((S[c, b2] + w - w2) > cap2).any():
                                continue
                            slot2node[c, sl], slot2node[c, sl2] = nd2, nd
                            S[c, b] = newb
                            S[c, b2] += w - w2
                            need -= w[dd] - w2[dd]
                            done = True
                            break
                        if done:
                            break
        # recompute improvements implicitly next round
        new_k = -(-np.maximum.reduce(S, 0) // BLK)
        improved = int(k.sum() - new_k.sum())
        if improved <= 0:
            break
    return slot2node


def preprocess(src, dst, et, ui, n, n_cores, sb_blocks=3, slot2node=None):
    """Bucket edges by (core, type, block); split by source half; build the
    shared chunk schedule plus per-core gather-index / one-hot-dst tensors."""
    shard, nblk, _ = _cfg(n, n_cores)
    if slot2node is None:
        core_of = dst // shard
        dloc = dst - core_of * shard
        blk = dloc // BLK
        doff = dloc - blk * BLK
    else:
        node2core = np.zeros(n, np.int64)
        node2pos = np.zeros(n, np.int64)
        for c in range(n_cores):
            s2n = slot2node[c]
            v = s2n >= 0
            node2core[s2n[v]] = c
            node2pos[s2n[v]] = np.where(v)[0]
        core_of = node2core[dst]
        pos = node2pos[dst]
        blk = pos // BLK
        doff = pos - blk * BLK

    tmap = np.full(3, -1, np.int64)
    tmap[BOUNDARY], tmap[REWIRE], tmap[UPPER] = 0, 1, 2
    t_of = tmap[et]

    key = (core_of * 3 + t_of) * nblk + blk
    order = np.argsort(key, kind="stable")
    key_s = key[order]
    src_s, doff_s, ui_s = src[order], doff[order], ui[order]
    starts = np.searchsorted(key_s, np.arange(n_cores * 3 * nblk + 1))

    def bucket(c, t, b):
        i0, i1 = starts[(c * 3 + t) * nblk + b], starts[(c * 3 + t) * nblk + b + 1]
        return i0, i1

    # per (core, chunk-type, block, half) edge (val, dst) lists
    # chunk types: 0=B (self+boundary), 1=R (self+rewire), 2=U1 (src),
    #              3=U2 (upper_ind); halves: 0=lo (<SPLIT), 1=hi
    lists = {}
    cnt = np.zeros((n_cores, NT, nblk, 2), np.int64)
    for c in range(n_cores):
        for b in range(nblk):
            i0, i1 = bucket(c, 0, b)
            vB, dB = src_s[i0:i1], doff_s[i0:i1]
            i0, i1 = bucket(c, 1, b)
            vR, dR = src_s[i0:i1], doff_s[i0:i1]
            i0, i1 = bucket(c, 2, b)
            vU1, dU1 = src_s[i0:i1], doff_s[i0:i1]
            vU2, dU2 = ui_s[i0:i1], doff_s[i0:i1]
            for t, (v, dd) in enumerate([(vB, dB), (vR, dR), (vU1, dU1), (vU2, dU2)]):
                o = np.argsort(v, kind="stable")   # ascending rows: HBM locality
                v, dd = v[o], dd[o]
                m = v < SPLIT
                lists[(c, t, b, 0)] = (v[m], dd[m])
                lists[(c, t, b, 1)] = (v[~m] - SPLIT, dd[~m])
                cnt[c, t, b, 0] = m.sum()
                cnt[c, t, b, 1] = (~m).sum()

    # shared schedule: chunks per (type, block, half); U1/U2 need >=1 chunk
    k = -(-cnt.max(axis=0) // BLK)          # [NT, nblk, 2]
    empty = k.sum(axis=2) == 0
    empty[0:2, :] = False                   # B/R init via identity matmul
    k[:, :, 0][empty] = 1

    # column layout: per super-block: [lo cols (b-major, t-minor)][hi cols]
    # first/last super-blocks kept small for faster pipeline fill/drain
    sizes = [1] + [sb_blocks] * ((nblk - 4) // sb_blocks) + [2, 1]
    sizes[1] += nblk - sum(sizes)
    sb_bounds = np.cumsum([0] + sizes).tolist()
    nsb_count = len(sb_bounds) - 1
    cols = {}            # (b, t) -> list of global slab/dst cols (lo then hi)
    sb_info = []         # per sb: (col0, Klo, Khi, gidx_lo0, gidx_hi0)
    col = 0
    gcol = 0
    for s in range(nsb_count):
        b0, b1 = sb_bounds[s], sb_bounds[s + 1]
        col0 = col
        for b in range(b0, b1):
            for t in range(NT):
                cols[(b, t)] = [col + j for j in range(int(k[t, b, 0]))]
                col += int(k[t, b, 0])
        Klo = col - col0
        for b in range(b0, b1):
            for t in range(NT):
                cols[(b, t)] = cols[(b, t)] + [col + j
                                               for j in range(int(k[t, b, 1]))]
                col += int(k[t, b, 1])
        Khi = col - col0 - Klo
        sb_info.append((col0, Klo, Khi, gcol, gcol + Klo * 8))
        gcol += (Klo + Khi) * 8
    K_TOT = col

    # gather pieces: (sb, half, col_lo, col_hi) — small pieces (<=8 chunks =
    # 1024 rows) so each fits comfortably in the SWDGE descriptor ring and
    # queue-rotation keeps several in flight without Q7 await-space stalls.
    pieces = []
    for s in range(nsb_count):
        col0, Klo, Khi, glo, ghi = sb_info[s]
        for half, Kh in [(0, Klo), (1, Khi)]:
            if Kh == 0:
                continue
            cut = 16
            for c0 in range(0, Kh, cut):
                pieces.append((s, half, c0, min(c0 + cut, Kh)))

    gidx = np.zeros((n_cores, BLK, K_TOT * 8), np.int16)
    # dst offsets duplicated x2 in the last dim: packed 2-byte pairs let the
    # DVE one-hot build (is_equal) run in its 2x/4x 16-bit perf mode.
    dst_t = np.full((n_cores, BLK, K_TOT, 2), PAD_DST, bf16)

    for c in range(n_cores):
        for s in range(nsb_count):
            b0, b1 = sb_bounds[s], sb_bounds[s + 1]
            col0, Klo, Khi, glo, ghi = sb_info[s]
            for half, Kh, g0, hoff in [(0, Klo, glo, 0), (1, Khi, ghi, Klo)]:
                if Kh == 0:
                    continue
                vals = np.zeros(Kh * BLK, np.int16)
                dsts = np.full(Kh * BLK, PAD_DST, np.float32)
                p = 0
                for b in range(b0, b1):
                    for t in range(NT):
                        kk = int(k[t, b, half])
                        if kk == 0:
                            continue
                        v, dd = lists[(c, t, b, half)]
                        vals[p:p + len(v)] = v
                        dsts[p:p + len(v)] = dd
                        # pad rows: cycle the segment's real rows (their
                        # one-hot is PAD->zero) so pads don't all hammer
                        # DRAM row 0
                        npad = kk * BLK - len(v)
                        if npad:
                            if len(v):
                                vals[p + len(v):p + kk * BLK] = np.resize(
                                    v, npad)
                            else:
                                bound = SPLIT if half == 0 else n - SPLIT
                                vals[p:p + kk * BLK] = (
                                    np.arange(kk * BLK) * 89) % bound
                        p += kk * BLK
                assert p == Kh * BLK
                # gather index i lives at partition i%16, col i//16 (x8 copies)
                gidx[c, :, g0:g0 + Kh * 8] = np.tile(
                    vals.reshape(-1, 16).T, (8, 1))
                dst_t[c, :, col0 + hoff:col0 + hoff + Kh, :] = (
                    dsts.reshape(Kh, BLK).T.astype(bf16)[:, :, None])

    sched = dict(k=k, cols=cols, sb_bounds=sb_bounds, sb_info=sb_info,
                 K_TOT=K_TOT, nblk=nblk, shard=shard, pieces=pieces)
    return gidx, dst_t, sched


def fuse_weights(p):
    f = np.float32
    W_uf = (p["umW"] @ p["uW1"]).astype(f)
    oW = p["oW"]
    Wb2o = (p["bW2"] @ oW[0:128]).astype(f)
    Wr2o = (p["rW2"] @ oW[128:256]).astype(f)
    Wu2o = (p["uW2"] @ oW[256:384]).astype(f)
    bu_f = (p["ub1"] + p["umb"] @ p["uW1"]).astype(f)
    ob_f = (p["ob"] + p["bb2"] @ oW[0:128] + p["rb2"] @ oW[128:256]
            + p["ub2"] @ oW[256:384]).astype(f)
    weights = np.concatenate(
        [p["bW1"], p["rW1"], p["uW1"], W_uf[:128], W_uf[128:],
         Wb2o, Wr2o, Wu2o], axis=1).astype(f)
    biases = np.stack([p["bb1"], p["rb1"], bu_f, ob_f], axis=1).astype(f)
    return weights, biases


# ---------------------------------------------------------------- kernel build
def build(sched, n):
    k, cols = sched["k"], sched["cols"]
    pieces = sched["pieces"]
    qrr = [0]
    sb_bounds, sb_info = sched["sb_bounds"], sched["sb_info"]
    K_TOT, nblk, shard = sched["K_TOT"], sched["nblk"], sched["shard"]
    shard_pad = nblk * BLK

    nc = bacc.Bacc(None, target_bir_lowering=False, debug=False,
                   num_swdge_queues=4)
    F32R = mybir.dt.float32r
    x16 = nc.dram_tensor("x16", [n, D], BF16, kind="ExternalInput")
    xsb = nc.dram_tensor("xsb", [BLK, nblk, D], BF16, kind="ExternalInput")
    xsbT = nc.dram_tensor("xsbT", [D, shard_pad], F32R, kind="ExternalInput")
    gidx = nc.dram_tensor("gidx", [BLK, K_TOT * 8], I16, kind="ExternalInput")
    dstv = nc.dram_tensor("dstv", [BLK, K_TOT, 2], BF16, kind="ExternalInput")
    wts = nc.dram_tensor("wts", [BLK, 8 * BLK], F32R, kind="ExternalInput")
    bia = nc.dram_tensor("bia", [BLK, 4], F32, kind="ExternalInput")
    iota = nc.dram_tensor("iota", [BLK, 64, 2], BF16, kind="ExternalInput")
    id16 = nc.dram_tensor("id16", [BLK, BLK], BF16, kind="ExternalInput")
    id32 = nc.dram_tensor("id32", [BLK, BLK], F32, kind="ExternalInput")
    outp = nc.dram_tensor("out", [shard_pad, D], BF16, kind="ExternalOutput")
    warm = nc.dram_tensor("warm", [BLK, D], BF16, kind="ExternalOutput")

    relu = mybir.ActivationFunctionType.Relu
    with TileContext(nc) as tc:
        with (
            tc.tile_pool(name="const", bufs=1) as cp,
            tc.tile_pool(name="gather", bufs=3) as gp,
            tc.tile_pool(name="slabs", bufs=2) as sp,
            tc.tile_pool(name="outs", bufs=4) as op,
            tc.tile_pool(name="psA", bufs=1, space="PSUM") as psA,
            tc.tile_pool(name="psB", bufs=1, space="PSUM") as psB,
        ):
            # SWDGE warmup: a dependency-free dummy gather (memset indices =
            # row 0) absorbs the one-time Q7/queue setup latency during the
            # const-load phase instead of stalling the first real gather.
            widx = cp.tile([BLK, 8], I16)
            nc.vector.memset(widx[:], 0)
            wslab = cp.tile([BLK, 1, D], BF16)
            nc.gpsimd.dma_gather(wslab[:, :, :], x16[0:SPLIT, :], widx[:, :],
                                 BLK, BLK, D, single_packet=False, queue_num=0)
            nc.sync.dma_start(out=warm[:, :], in_=wslab[:, 0, :])

            wts_s = cp.tile([BLK, 8 * BLK], F32R)
            nc.sync.dma_start(out=wts_s[:], in_=wts[:, :])
            bia_s = cp.tile([BLK, 4], F32)
            nc.sync.dma_start(out=bia_s[:], in_=bia[:, :])
            iota_s = cp.tile([BLK, 64, 2], BF16)
            nc.sync.dma_start(out=iota_s[:], in_=iota[:, :, :])
            id16_s = cp.tile([BLK, BLK], BF16)
            nc.sync.dma_start(out=id16_s[:], in_=id16[:, :])
            id32_s = cp.tile([BLK, BLK], F32)
            nc.sync.dma_start(out=id32_s[:], in_=id32[:, :])

            w = {nm: wts_s[:, i * BLK:(i + 1) * BLK]
                 for i, nm in enumerate(["bW1", "rW1", "uW1", "W_uf_a", "W_uf_b",
                                         "Wb2o", "Wr2o", "Wu2o"])}

            for s in range(len(sb_bounds) - 1):
                b0, b1 = sb_bounds[s], sb_bounds[s + 1]
                nb = b1 - b0
                nsb = nb * BLK
                col0, Klo, Khi, glo, ghi = sb_info[s]
                Ks = Klo + Khi

                # just-in-time gidx/dst slices for this super-block
                # (Activation HWDGE queue, off the gather critical path)
                gidx_s = gp.tile([BLK, Ks * 8], I16, tag="gidx")
                nc.scalar.dma_start(out=gidx_s[:],
                                    in_=gidx[:, glo:glo + Ks * 8])
                dst_s = gp.tile([BLK, Ks, 2], BF16, tag="dst")
                nc.scalar.dma_start(out=dst_s[:],
                                    in_=dstv[:, col0:col0 + Ks, :])

                slab = gp.tile([BLK, Ks, D], BF16, tag="slab")
                for (ps_, half, c0, c1) in pieces:
                    if ps_ != s:
                        continue
                    npc = (c1 - c0) * BLK
                    if half == 0:
                        nc.gpsimd.dma_gather(
                            slab[:, c0:c1, :], x16[0:min(SPLIT, n), :],
                            gidx_s[:, c0 * 8:c1 * 8],
                            npc, npc, D, single_packet=False,
                            queue_num=qrr[0] % 4)
                    else:
                        nc.gpsimd.dma_gather(
                            slab[:, Klo + c0:Klo + c1, :], x16[SPLIT:n, :],
                            gidx_s[:, (Klo + c0) * 8:(Klo + c1) * 8],
                            npc, npc, D, single_packet=False,
                            queue_num=qrr[0] % 4)
                    qrr[0] += 1
                xsl = gp.tile([BLK, nb, D], BF16, tag="xsl")
                nc.sync.dma_start(out=xsl[:], in_=xsb[:, b0:b1, :])
                xT_s = sp.tile([D, nsb], F32R, tag="xT")
                nc.sync.dma_start(out=xT_s[:],
                                  in_=xsbT[:, b0 * BLK:b0 * BLK + nsb])
                # one-hot dst matrix; packed 2-wide last dim keeps DVE in its
                # 16-bit high-throughput mode
                A = gp.tile([BLK, Ks, 64, 2], BF16, tag="A")
                nc.vector.tensor_tensor(
                    out=A[:],
                    in0=iota_s[:, None, :, :].to_broadcast([BLK, Ks, 64, 2]),
                    in1=dst_s[:, :, None, :].to_broadcast([BLK, Ks, 64, 2]),
                    op=mybir.AluOpType.is_equal,
                )

                ps = {t: psA.tile([D, nsb], F32, tag=f"ps_{t}", name=f"ps_{t}_{s}")
                      for t in range(NT)}

                for b in range(b0, b1):
                    bc = b - b0
                    sl = bass.ts(bc, BLK)
                    for t in range(NT):
                        cl = cols[(b, t)]
                        if t < 2:
                            # x term of (1+eps)*x + agg rides as an identity MM
                            nc.tensor.matmul(out=ps[t][:, sl],
                                             lhsT=xsl[:, bc, :],
                                             rhs=id16_s[:, :],
                                             start=True, stop=(not cl))
                        for j, cg in enumerate(cl):
                            lc = cg - col0
                            nc.tensor.matmul(out=ps[t][:, sl],
                                             lhsT=slab[:, lc, :],
                                             rhs=A[:, lc, :, :],
                                             start=(t >= 2 and j == 0),
                                             stop=(j == len(cl) - 1))

                b_s = sp.tile([D, nsb], F32R, tag="b_s")
                nc.scalar.copy(out=b_s[:], in_=ps[0][:])
                r_s = sp.tile([D, nsb], F32R, tag="r_s")
                nc.scalar.copy(out=r_s[:], in_=ps[1][:])
                u1_s = sp.tile([D, nsb], F32R, tag="u1_s")
                nc.vector.tensor_copy(out=u1_s[:], in_=ps[2][:])
                u2_s = sp.tile([D, nsb], F32R, tag="u2_s")
                nc.vector.tensor_copy(out=u2_s[:], in_=ps[3][:])

                def mm(out, wn, rhs, start, stop):
                    nc.tensor.matmul(out=out, lhsT=w[wn], rhs=rhs,
                                     start=start, stop=stop)

                h1b_p = psB.tile([D, nsb], F32, tag="h1")
                mm(h1b_p[:], "bW1", b_s[:], True, True)
                h1b_s = sp.tile([D, nsb], F32R, tag="h1b")
                nc.scalar.activation(out=h1b_s[:], in_=h1b_p[:], func=relu,
                                     bias=bia_s[:, 0:1])
                h1r_p = psB.tile([D, nsb], F32, tag="h1")
                mm(h1r_p[:], "rW1", r_s[:], True, True)
                h1r_s = sp.tile([D, nsb], F32R, tag="h1r")
                nc.scalar.activation(out=h1r_s[:], in_=h1r_p[:], func=relu,
                                     bias=bia_s[:, 1:2])
                h1u_p = psB.tile([D, nsb], F32, tag="h1")
                mm(h1u_p[:], "uW1", xT_s[:], True, False)
                mm(h1u_p[:], "W_uf_a", u1_s[:], False, False)
                mm(h1u_p[:], "W_uf_b", u2_s[:], False, True)
                h1u_s = sp.tile([D, nsb], F32R, tag="h1u")
                nc.scalar.activation(out=h1u_s[:], in_=h1u_p[:], func=relu,
                                     bias=bia_s[:, 2:3])

                out_p = psB.tile([D, nsb], F32, tag="outp")
                mm(out_p[:], "Wb2o", h1b_s[:], True, False)
                mm(out_p[:], "Wr2o", h1r_s[:], False, False)
                mm(out_p[:], "Wu2o", h1u_s[:], False, True)
                outT_s = sp.tile([D, nsb], F32, tag="outT")
                nc.scalar.activation(out=outT_s[:], in_=out_p[:], func=relu,
                                     bias=bia_s[:, 3:4])

                for bc in range(nb):
                    tr_p = psB.tile([BLK, BLK], F32, tag="tr")
                    nc.tensor.transpose(out=tr_p[:],
                                        in_=outT_s[:, bass.ts(bc, BLK)],
                                        identity=id32_s[:, :])
                    onat = op.tile([BLK, BLK], BF16, tag="onat")
                    if bc % 2 == 0:
                        nc.vector.tensor_copy(out=onat[:], in_=tr_p[:])
                    else:
                        nc.scalar.copy(out=onat[:], in_=tr_p[:])
                    nc.sync.dma_start(out=outp[(b0 + bc) * BLK:(b0 + bc + 1) * BLK, :],
                                      in_=onat[:])
    nc.compile()
    return nc


# ---------------------------------------------------------------- entry point
def kernel(x, edge_index, edge_type, upper_ind, cell_dimension,
           bW1, bb1, bW2, bb2, rW1, rb1, rW2, rb2,
           umW, umb, uW1, ub1, uW2, ub2, oW, ob, _trace=False):
    global LAST_EXEC_NS, LAST_TRACE_PATH
    params = dict(bW1=bW1, bb1=bb1, bW2=bW2, bb2=bb2, rW1=rW1, rb1=rb1,
                  rW2=rW2, rb2=rb2, umW=umW, umb=umb, uW1=uW1, ub1=ub1,
                  uW2=uW2, ub2=ub2, oW=oW, ob=ob)
    params = {k_: np.asarray(v, np.float32) for k_, v in params.items()}
    x = np.asarray(x, np.float32)
    src = np.asarray(edge_index[0], np.int64)
    dst = np.asarray(edge_index[1], np.int64)
    et = np.asarray(edge_type, np.int64)
    ui = np.asarray(upper_ind, np.int64)

    shard, nblk, shard_pad = _cfg(N, NC)
    d8 = node_degrees(src, dst, et, ui, N)
    slot2node = assign_nodes(d8, NC, nblk)
    gidx_t, dst_t, sched = preprocess(src, dst, et, ui, N, NC,
                                      slot2node=slot2node)
    weights, biases = fuse_weights(params)

    x16 = x.astype(bf16)
    xsb_all = []
    xsbT_all = []
    for c in range(NC):
        rows = np.where(slot2node[c] >= 0, slot2node[c], 0)
        xsb_all.append(np.ascontiguousarray(
            x16[rows].reshape(nblk, BLK, D).transpose(1, 0, 2)))
        xsbT_all.append(np.ascontiguousarray(x[rows].T))

    iota_np = np.broadcast_to(np.arange(BLK, dtype=np.float32), (BLK, BLK)
                              ).astype(bf16).reshape(BLK, 64, 2)
    ident = np.eye(BLK, dtype=np.float32)

    nc = build(sched, N)

    in_maps = []
    for c in range(NC):
        in_maps.append({
            "x16": x16, "xsb": xsb_all[c], "xsbT": xsbT_all[c],
            "gidx": gidx_t[c], "dstv": dst_t[c],
            "wts": weights, "bia": biases, "iota": np.ascontiguousarray(iota_np),
            "id16": ident.astype(bf16), "id32": ident,
        })
    res = run_bass_kernel_spmd(nc, in_maps, core_ids=list(range(NC)),
                               trace=_trace, trace_cores=list(range(NC)))
    LAST_EXEC_NS = res.exec_time_ns
    if res.instructions_and_trace is not None:
        LAST_TRACE_PATH = res.instructions_and_trace[1]
    out = np.empty((N, D), np.float32)
    for c in range(NC):
        s2n = slot2node[c]
        v = s2n >= 0
        out[s2n[v]] = res.results[c]["out"][v]
    return out


if __name__ == "__main__":
    import reference
    inp = {k_: np.asarray(v) for k_, v in reference.setup_inputs().items()}
    got = kernel(**inp)
    exp = np.asarray(reference.reference(**inp))
    print(f"Relative error: {np.linalg.norm(got - exp) / np.linalg.norm(exp):.4e}")

